# revision 18
# baseline (speedup 1.0000x reference)
"""Trainium2 Bass kernel for the periodic flux-divergence stencil:

    out = sum_ax  (v - roll(v, 1, ax)),  v = 0.5*(roll(M,-1,ax)+M)*(roll(mu,-1,ax)-mu)

over axes H, W of [B=16, 1, 1024, 1024] f32 inputs, data-parallel over batch
across 8 NeuronCores (2 images per core).

fp16 + software-pipelined design:
  - host converts inputs to fp16 (halves HBM traffic, doubles DVE rate),
    pre-scales M by 0.5, pads each image row with circular wrap columns
  - 9 row-blocks of 128 rows advancing by 126 (overlap absorbs H-halo)
  - the loop is skewed into 3 stages so every engine always has ready work:
      S0(t): DMA loads (SP queue)
      S1(t): DVE aw/gw/vwu; PE psA=A@m, psD=F@mu; ACT dh=copy(psD);
             DVE vhu = psA*dh
      S2(t): PE psC = L@vhu + I@vwuR + (-I)@vwuL  (H-div + W-div folded);
             ACT out = copy(psC); GPSIMD store (SWDGE queue)
"""
import sys

sys.path.insert(0, "/opt/trn_rl_repo")

import numpy as np

B, H, W = 16, 1024, 1024
N_CORES = 8
IMGS_PER_CORE = B // N_CORES          # 2
PW = W + 2                            # padded row width (wrap cols)
ADV = 126                             # valid rows per block
NBLK = (H + ADV - 1) // ADV           # 9
HALF = 512                            # PSUM bank width (f32)
EXT = ADV * (NBLK - 1) + 128          # 1136: extended rows so every block
                                      # load is one aligned [128,:] DMA

_CACHE = {}


def _build():
    import concourse.mybir as mybir
    from concourse import bacc
    from concourse.tile import TileContext

    f16 = mybir.dt.float16
    f32 = mybir.dt.float32
    Alu = mybir.AluOpType

    nc = bacc.Bacc(trn_type="TRN2", target_bir_lowering=False)

    M_d = nc.dram_tensor("m_in", [EXT, 2 * PW], f16, kind="ExternalInput")
    MU_d = nc.dram_tensor("mu_in", [EXT, 2 * PW], f16, kind="ExternalInput")
    ST_d = nc.dram_tensor("stencils", [128, 5 * 128], f16, kind="ExternalInput")
    OUT_d = nc.dram_tensor("out", [H, 2 * W], f16, kind="ExternalOutput")

    with TileContext(nc) as tc:
        with (
            tc.tile_pool(name="consts", bufs=1) as cpool,
            tc.tile_pool(name="io", bufs=4) as iopool,
            tc.tile_pool(name="work", bufs=2) as wpool,
            tc.tile_pool(name="keep", bufs=4) as kpool,
            tc.tile_pool(name="psA", bufs=1, space="PSUM") as poolA,
            tc.tile_pool(name="psD", bufs=1, space="PSUM") as poolD,
            tc.tile_pool(name="psC", bufs=1, space="PSUM") as poolC,
        ):
            st = cpool.tile([128, 5 * 128], f16)
            nc.sync.dma_start(out=st[:], in_=ST_d[:])
            lA = st[:, 0:128]      # (I+U).T      H forward-average (x2)
            lF = st[:, 128:256]    # (U-I).T      H forward-diff
            lL = st[:, 256:384]    # (I-D).T      H backward-diff (divergence)
            lI = st[:, 384:512]    # I            W-part fold (+vwu_right)
            lIn = st[:, 512:640]   # -I           W-part fold (-vwu_left)

            tiles = {}

            # moving-slice starts: (pair, half) -> fp16 input col, out col
            def mcol(p, hh):
                img, colh = divmod(2 * p + hh, 2)
                return img * PW + 1 + colh * HALF, img * W + colh * HALF

            def S0(t):
                r0 = ADV * t
                mu_t = iopool.tile([128, 2 * PW], f16, tag="mu", name="mu_t")
                m_t = iopool.tile([128, 2 * PW], f16, tag="m", name="m_t")
                nc.sync.dma_start(out=mu_t[:], in_=MU_d[r0:r0 + 128, :])
                nc.sync.dma_start(out=m_t[:], in_=M_d[r0:r0 + 128, :])
                tiles[t] = {"mu": mu_t, "m": m_t}

            def S1(t):
                d = tiles[t]
                mu_t, m_t = d["mu"], d["m"]
                m3 = m_t[:].rearrange("p (j k) -> p j k", j=2)
                mu3 = mu_t[:].rearrange("p (j k) -> p j k", j=2)

                aw = wpool.tile([128, 2 * (W + 1)], f16, tag="aw", name="aw")
                aw3 = aw[:].rearrange("p (j k) -> p j k", j=2)
                nc.vector.tensor_tensor(
                    out=aw3, in0=m3[:, :, 0:W + 1], in1=m3[:, :, 1:W + 2],
                    op=Alu.add)
                gw = wpool.tile([128, 2 * (W + 1)], f16, tag="gw", name="gw")
                gw3 = gw[:].rearrange("p (j k) -> p j k", j=2)
                nc.vector.tensor_tensor(
                    out=gw3, in0=mu3[:, :, 1:W + 2], in1=mu3[:, :, 0:W + 1],
                    op=Alu.subtract)
                vwu = kpool.tile([128, 2 * (W + 1)], f16, tag="vwu", name="vwu")
                nc.vector.tensor_tensor(
                    out=vwu[:], in0=aw[:], in1=gw[:], op=Alu.mult)

                # psA: one 4-bank megatile; A-matmuls share one LDWEIGHTS
                psA = poolA.tile([128, 4 * HALF], f32, tag="psA", name="psA")
                for p in range(2):
                    for hh in range(2):
                        c0, o0 = mcol(p, hh)
                        nc.tensor.matmul(
                            psA[:, o0:o0 + HALF], lA,
                            m_t[:, c0:c0 + HALF], start=True, stop=True)
                dh = wpool.tile([128, 2 * W], f16, tag="dh", name="dh")
                for p in range(2):
                    psD = poolD.tile([128, 2 * HALF], f32, tag="psD", name="psD")
                    for hh in range(2):
                        c0, _ = mcol(p, hh)
                        nc.tensor.matmul(
                            psD[:, hh * HALF:(hh + 1) * HALF], lF,
                            mu_t[:, c0:c0 + HALF], start=True, stop=True)
                    nc.scalar.copy(
                        out=dh[:, p * 1024:(p + 1) * 1024], in_=psD[:])
                vhu = kpool.tile([128, 2 * W], f16, tag="vhu", name="vhu")
                nc.vector.tensor_tensor(
                    out=vhu[:], in0=psA[:], in1=dh[:], op=Alu.mult)
                d["vwu"] = vwu
                d["vhu"] = vhu

            def S2(t):
                d = tiles[t]
                vwu, vhu = d["vwu"], d["vhu"]
                out_t = kpool.tile([128, 2 * W], f16, tag="out", name="out_t")
                for p in range(2):
                    psC = poolC.tile([128, 2 * HALF], f32, tag="psC", name="psC")
                    for hh in range(2):
                        _, o0 = mcol(p, hh)
                        nc.tensor.matmul(
                            psC[:, hh * HALF:(hh + 1) * HALF], lL,
                            vhu[:, o0:o0 + HALF], start=True, stop=False)
                    for hh in range(2):
                        img, colh = divmod(2 * p + hh, 2)
                        cR = img * (W + 1) + colh * HALF + 1
                        nc.tensor.matmul(
                            psC[:, hh * HALF:(hh + 1) * HALF], lI,
                            vwu[:, cR:cR + HALF], start=False, stop=False)
                    for hh in range(2):
                        img, colh = divmod(2 * p + hh, 2)
                        cL = img * (W + 1) + colh * HALF
                        nc.tensor.matmul(
                            psC[:, hh * HALF:(hh + 1) * HALF], lIn,
                            vwu[:, cL:cL + HALF], start=False, stop=True)
                    nc.scalar.copy(
                        out=out_t[:, p * 1024:(p + 1) * 1024], in_=psC[:])

                r_out = ADV * t
                nvalid = min(ADV, H - r_out)
                nc.gpsimd.dma_start(
                    out=OUT_d[r_out:r_out + nvalid, :],
                    in_=out_t[1:1 + nvalid, :])
                del tiles[t]

            for r in range(NBLK + 2):
                if r < NBLK:
                    S0(r)
                if r >= 2:
                    S2(r - 2)
                if 1 <= r <= NBLK:
                    S1(r - 1)

    nc.compile()
    return nc


def _stencils():
    A = np.zeros((128, 128), dtype=np.float32)
    F = np.zeros((128, 128), dtype=np.float32)
    L = np.zeros((128, 128), dtype=np.float32)
    for r in range(127):
        A[r, r] = 1.0
        A[r, r + 1] = 1.0
        F[r, r] = -1.0
        F[r, r + 1] = 1.0
    A[127, 127] = 1.0
    F[127, 127] = -1.0
    for r in range(128):
        L[r, r] = 1.0
    for r in range(1, 128):
        L[r, r - 1] = -1.0
    st = np.zeros((128, 5 * 128), dtype=np.float32)
    st[:, 0:128] = A.T
    st[:, 128:256] = F.T
    st[:, 256:384] = L.T
    st[:, 384:512] = np.eye(128, dtype=np.float32)
    st[:, 512:640] = -np.eye(128, dtype=np.float32)
    return st.astype(np.float16)


def _pad_rows(x):
    """[2, H, W] fp16 -> [EXT, 2*(W+2)]: circular wrap columns + row r holds
    original row (r-1) mod H so block loads need no wrap splits."""
    out = np.empty((H, 2, PW), dtype=np.float16)
    for j in range(2):
        out[:, j, 1:W + 1] = x[j]
        out[:, j, 0] = x[j][:, W - 1]
        out[:, j, W + 1] = x[j][:, 0]
    flat = out.reshape(H, 2 * PW)
    idx = (np.arange(EXT) - 1) % H
    return np.ascontiguousarray(flat[idx])


def make_in_maps(inputs):
    M = np.asarray(inputs["M"], dtype=np.float32).reshape(B, H, W)
    mu = np.asarray(inputs["mu"], dtype=np.float32).reshape(B, H, W)
    st = _stencils()
    in_maps = []
    for c in range(N_CORES):
        i0 = c * IMGS_PER_CORE
        ms = (M[i0:i0 + 2] * 0.5).astype(np.float16)
        mus = mu[i0:i0 + 2].astype(np.float16)
        in_maps.append({
            "m_in": _pad_rows(ms),
            "mu_in": _pad_rows(mus),
            "stencils": st,
        })
    return in_maps


def kernel(M, mu):
    from concourse.bass_utils import run_bass_kernel_spmd

    if "nc" not in _CACHE:
        _CACHE["nc"] = _build()
    nc = _CACHE["nc"]

    in_maps = make_in_maps({"M": M, "mu": mu})

    res = run_bass_kernel_spmd(nc, in_maps, core_ids=list(range(N_CORES)))
    out = np.empty((B, H, W), dtype=np.float32)
    for c in range(N_CORES):
        o = res.results[c]["out"].reshape(H, 2, W)
        for j in range(IMGS_PER_CORE):
            out[c * IMGS_PER_CORE + j] = o[:, j, :].astype(np.float32)
    return out.reshape(B, 1, H, W)
